# revision 10
# baseline (speedup 1.0000x reference)
"""Trainium2 Bass kernel for nn_AttentionBlock_15693810500077.

GroupNorm(32 groups) -> 1x1 qkv conv -> 4-head attention (T=4096) ->
1x1 proj -> residual, for x [2, 256, 16, 16, 16] fp32.

Sharding: 8 cores = (batch b in {0,1}) x (t-slice i in {0..3}, TS=1024).
Each core computes the full attention rows for its t-slice of its batch,
for all 4 heads, plus the projection and residual -> y^T slab [1024, 256].
The host rotates each core's x copy (np.roll over T) so the core's t-slice
always sits at columns 0:1024 -> one static SPMD program for all cores
(softmax over keys is permutation invariant).

Self-contained: hardcodes all shapes; only needs numpy + the concourse
(Bass) runtime available in the environment.
"""
import os

import numpy as np

os.environ.setdefault("JAX_COMPILATION_CACHE_DIR", "/tmp/jaxcache")

import concourse.bass as bass
import concourse.bacc as bacc
import concourse.tile as tile
from concourse import mybir
from concourse.vector_clock import ScopedClock
from concourse.bass_utils import run_bass_kernel_spmd

F32 = mybir.dt.float32
F32R = mybir.dt.float32r
AF = mybir.ActivationFunctionType
ALU = mybir.AluOpType

H = 4
C = 256
T = 4096
TS = 1024
EPS = 1e-5
SCALE2 = 0.125           # (1/sqrt(sqrt(64)))^2
NCHUNKS = T // 128       # 32 key chunks of 128


class _SplitDrainTileContext(tile.TileContext):
    """Walrus in this env rejects >1 sync wait on an SP Drain; split the
    kernel-tail drain's waits across a chain of single-wait drains."""

    def _drain_and_barrier(self, tick_clock, wait_clock):
        drain_inst = self.nc.sync.drain()
        wait_clock.add_sem_waits(
            drain_inst.ins, ScopedClock({None: tick_clock.global_clock})
        )
        si = drain_inst.ins.sync_info
        waits = list(si.on_wait or []) if si is not None else []
        if len(waits) > 1:
            si.on_wait = waits[:1]
            for j in range(1, len(waits)):
                d2 = self.nc.sync.drain()
                d2.ins.sync_info = mybir.SyncInfo(
                    on_wait=waits[j : j + 1], on_update=[]
                )
        self.nc.all_engine_barrier()
        assert self.sems is not None
        popped = self.nc._tile_sem_poison_stack.pop()
        assert popped is self._sem_poison
        self.nc.clear_and_free_semaphores(list(self.sems.allocated().values()))
        self.nc.all_engine_barrier()


def _mm(nc, out, lhsT, rhs, start=True, stop=True, r=True):
    """matmul with fp32r bitcast and N<=512 chunking along the free dim."""
    n = rhs.free_size()
    lt = lhsT.bitcast(F32R) if r else lhsT
    for n0 in range(0, n, 512):
        n1 = min(n0 + 512, n)
        rh = rhs[:, n0:n1]
        nc.tensor.matmul(
            out[:, n0:n1],
            lt,
            rh.bitcast(F32R) if r else rh,
            start=start,
            stop=stop,
        )


def build_nc():
    nc = bass.Bass()

    x_d = nc.dram_tensor("x", [C, T], F32, kind="ExternalInput")
    xT_d = nc.dram_tensor("xT", [TS, C], F32, kind="ExternalInput")
    wqT_d = nc.dram_tensor("wqT", [C, C], F32R, kind="ExternalInput")
    wkT_d = nc.dram_tensor("wkT", [C, C], F32R, kind="ExternalInput")
    wvT_d = nc.dram_tensor("wvT", [C, C], F32R, kind="ExternalInput")
    pT_d = nc.dram_tensor("pT", [4, 64, C], F32R, kind="ExternalInput")
    normw_d = nc.dram_tensor("normw", [2, 128, 1], F32, kind="ExternalInput")
    normb_d = nc.dram_tensor("normb", [2, 128, 1], F32, kind="ExternalInput")
    projb_d = nc.dram_tensor("projb", [1, C], F32R, kind="ExternalInput")
    sel_d = nc.dram_tensor("sel", [128, 16], F32, kind="ExternalInput")
    exp_d = nc.dram_tensor("expand", [16, 128], F32, kind="ExternalInput")
    ones_d = nc.dram_tensor("ones", [128, 128], F32R, kind="ExternalInput")
    yT_d = nc.dram_tensor("yT", [TS, C], F32, kind="ExternalOutput")

    import contextlib

    with tile.TileContext(nc) as tc:
        with (
            tc.tile_pool(name="consts", bufs=1) as consts,
            tc.tile_pool(name="gnp", bufs=2) as gnp,
            tc.tile_pool(name="kqv", bufs=1) as kqv,
            tc.tile_pool(name="psA", bufs=2, space="PSUM") as psA,
            tc.tile_pool(name="psB", bufs=2, space="PSUM") as psB,
            contextlib.ExitStack() as late,
        ):
            # ---- constant / weight loads ----
            wq = [consts.tile([128, C], F32R, name=f"wq{i}") for i in range(2)]
            wk = [consts.tile([128, C], F32R, name=f"wk{i}") for i in range(2)]
            wv = [consts.tile([128, C], F32R, name=f"wv{i}") for i in range(2)]
            for i in range(2):
                nc.sync.dma_start(out=wq[i], in_=wqT_d[i * 128:(i + 1) * 128, :])
                nc.sync.dma_start(out=wk[i], in_=wkT_d[i * 128:(i + 1) * 128, :])
                nc.sync.dma_start(out=wv[i], in_=wvT_d[i * 128:(i + 1) * 128, :])
            pT = [consts.tile([64, C], F32R, name=f"pT{h}") for h in range(H)]
            for h in range(H):
                nc.sync.dma_start(out=pT[h], in_=pT_d[h])
            normw = [consts.tile([128, 1], F32, name=f"nw{i}") for i in range(2)]
            normb = [consts.tile([128, 1], F32, name=f"nb{i}") for i in range(2)]
            for i in range(2):
                nc.sync.dma_start(out=normw[i], in_=normw_d[i])
                nc.sync.dma_start(out=normb[i], in_=normb_d[i])
            projb = consts.tile([1, C], F32R, name="projb")
            nc.sync.dma_start(out=projb, in_=projb_d[:])
            sel = consts.tile([128, 16], F32, name="sel")
            nc.sync.dma_start(out=sel, in_=sel_d[:])
            expand = consts.tile([16, 128], F32, name="expand")
            nc.sync.dma_start(out=expand, in_=exp_d[:])
            xT_sb = consts.tile([128, 8, C], F32, name="xT_sb")
            nc.sync.dma_start(
                out=xT_sb, in_=xT_d.rearrange("(a p) o -> p a o", p=128)
            )
            ones = consts.tile([128, 128], F32R, name="ones")
            nc.sync.dma_start(out=ones, in_=ones_d[:])

            # ---- load x, GroupNorm -> xn ----
            xn = [kqv.tile([128, T], F32R, name=f"xn{i}") for i in range(2)]
            with tc.tile_pool(name="xp", bufs=1) as xp:
                xt = [xp.tile([128, T], F32, name=f"x{i}") for i in range(2)]
                for i in range(2):
                    nc.sync.dma_start(out=xt[i], in_=x_d[i * 128:(i + 1) * 128, :])
                for i in range(2):
                    xv = xt[i].rearrange("p (a f) -> p a f", f=512)
                    stats = gnp.tile([128, 8, 6], F32, name="stats", tag="stats")
                    for j in range(8):
                        nc.vector.bn_stats(out=stats[:, j, :], in_=xv[:, j, :])
                    mv = gnp.tile([128, 2], F32, name="mv", tag="mv")
                    nc.vector.bn_aggr(out=mv, in_=stats)
                    # exsq = var + mean^2
                    msq = gnp.tile([128, 1], F32, name="msq", tag="msq")
                    nc.vector.tensor_mul(msq, mv[:, 0:1], mv[:, 0:1])
                    exsq = gnp.tile([128, 1], F32, name="exsq", tag="exsq")
                    nc.vector.tensor_add(exsq, msq, mv[:, 1:2])
                    # group stats via selector matmuls (plain fp32, tiny)
                    gm_ps = psB.tile([16, 1], F32, name="gm_ps", tag="acc")
                    nc.tensor.matmul(gm_ps, sel, mv[:, 0:1], start=True, stop=True)
                    gx_ps = psB.tile([16, 1], F32, name="gx_ps", tag="acc")
                    nc.tensor.matmul(gx_ps, sel, exsq, start=True, stop=True)
                    gm_sb = gnp.tile([16, 1], F32, name="gm_sb", tag="gm_sb")
                    nc.vector.tensor_copy(gm_sb, gm_ps)
                    gmsq = gnp.tile([16, 1], F32, name="gmsq", tag="gmsq")
                    nc.vector.tensor_mul(gmsq, gm_sb, gm_sb)
                    gvar = gnp.tile([16, 1], F32, name="gvar", tag="gvar")
                    nc.vector.scalar_tensor_tensor(
                        gvar, gx_ps, EPS, gmsq, op0=ALU.add, op1=ALU.subtract
                    )
                    # rstd = exp(-0.5 * ln(var + eps))
                    lnv = gnp.tile([16, 1], F32, name="lnv", tag="lnv")
                    nc.scalar.activation(lnv, gvar, AF.Ln)
                    rstd = gnp.tile([16, 1], F32, name="rstd", tag="rstd")
                    nc.scalar.activation(rstd, lnv, AF.Exp, scale=-0.5)
                    # expand to channels
                    me_ps = psB.tile([128, 1], F32, name="me_ps", tag="acc")
                    nc.tensor.matmul(me_ps, expand, gm_sb, start=True, stop=True)
                    re_ps = psB.tile([128, 1], F32, name="re_ps", tag="acc")
                    nc.tensor.matmul(re_ps, expand, rstd, start=True, stop=True)
                    a_sb = gnp.tile([128, 1], F32, name="a_sb", tag="a_sb")
                    nc.vector.tensor_mul(a_sb, re_ps, normw[i])
                    t2 = gnp.tile([128, 1], F32, name="t2", tag="t2")
                    nc.vector.tensor_mul(t2, me_ps, a_sb)
                    b_sb = gnp.tile([128, 1], F32, name="b_sb", tag="b_sb")
                    nc.vector.tensor_sub(b_sb, normb[i], t2)
                    nc.vector.tensor_scalar(
                        out=xn[i], in0=xt[i], scalar1=a_sb, scalar2=b_sb,
                        op0=ALU.mult, op1=ALU.add,
                    )

            # ---- late pools (opened after the x pool is released) ----
            ppool = late.enter_context(tc.tile_pool(name="ppool", bufs=3))
            rsp = late.enter_context(tc.tile_pool(name="rsp", bufs=2))
            stk = late.enter_context(tc.tile_pool(name="stk", bufs=1))
            outp = late.enter_context(tc.tile_pool(name="outp", bufs=1))

            # ---- qkv ----
            q_sb = [kqv.tile([128, TS], F32R, name=f"q{o}") for o in range(2)]
            k_sb = [kqv.tile([128, T], F32R, name=f"k{o}") for o in range(2)]
            vTa = kqv.tile([128, H, NCHUNKS, 65], F32R, name="vTa")
            nc.sync.dma_start(
                out=vTa[:, :, :, 64:65],
                in_=ones_d.rearrange("p (a b one) -> p a b one", a=H, one=1),
            )
            for o in range(2):
                q_ps = psA.tile([128, TS], F32, name="q_ps", tag="big")
                for cc in range(2):
                    _mm(nc, q_ps, wq[cc][:, o * 128:(o + 1) * 128],
                        xn[cc][:, 0:TS], start=(cc == 0), stop=(cc == 1))
                nc.vector.tensor_copy(q_sb[o], q_ps)
            for o in range(2):
                for nk in range(8):
                    k_ps = psA.tile([128, 512], F32, name="k_ps", tag="big")
                    for cc in range(2):
                        _mm(nc, k_ps, wk[cc][:, o * 128:(o + 1) * 128],
                            xn[cc][:, nk * 512:(nk + 1) * 512],
                            start=(cc == 0), stop=(cc == 1))
                    nc.vector.tensor_copy(k_sb[o][:, nk * 512:(nk + 1) * 512], k_ps)
            for tci in range(NCHUNKS):
                vt_ps = psA.tile([128, C], F32, name="vt_ps", tag="big")
                for cc in range(2):
                    _mm(nc, vt_ps, xn[cc][:, tci * 128:(tci + 1) * 128],
                        wv[cc], start=(cc == 0), stop=(cc == 1))
                nc.vector.tensor_copy(
                    vTa[:, :, tci, 0:64],
                    vt_ps.rearrange("p (h c) -> p h c", h=H),
                )

            # ---- attention (head pairs share k/q tiles; S^T layout) ----
            stacks = {}
            for pair in ((0, 1), (2, 3)):
                pv_ps = {}
                for h in pair:
                    pv_ps[h] = psB.tile([65, TS], F32, name=f"pv{h}", tag="acc")
                for sc in range(NCHUNKS):
                    p_t = {}
                    for h in pair:
                        kt = k_sb[h // 2]
                        qt = q_sb[h // 2]
                        lo = (h % 2) * 64
                        qk_ps = psA.tile([128, TS], F32, name="qk_ps", tag="big")
                        _mm(nc, qk_ps,
                            kt[lo:lo + 64, sc * 128:(sc + 1) * 128],
                            qt[lo:lo + 64, :])
                        p_t[h] = ppool.tile([128, TS], F32R, name="p_t", tag="p")
                        nc.scalar.activation(p_t[h], qk_ps, AF.Exp, scale=SCALE2)
                    for h in pair:
                        _mm(nc, pv_ps[h], vTa[:, h, sc, :], p_t[h],
                            start=(sc == 0), stop=(sc == NCHUNKS - 1))
                # normalize: stack_h = out2 / rowsum
                for h in pair:
                    rs_sb = rsp.tile([65, TS], F32R, name="rs_sb", tag="rs")
                    nc.scalar.copy(rs_sb[64:65, :], pv_ps[h][64:65, :])
                    bc_ps = psA.tile([64, TS], F32, name="bc_ps", tag="big")
                    _mm(nc, bc_ps, ones[64:65, 0:64], rs_sb[64:65, :])
                    recip = rsp.tile([64, TS], F32, name="recip", tag="recip")
                    nc.vector.reciprocal(recip, bc_ps)
                    stack = stk.tile([64, TS], F32R, name=f"stack{h}",
                                     tag=f"stack{h}")
                    nc.vector.tensor_mul(stack, pv_ps[h][0:64, :], recip)
                    stacks[h] = stack

            # ---- proj + bias + residual ----
            out_sb = outp.tile([128, 8, C], F32, name="out_sb")
            for tci in range(8):
                pr_ps = psB.tile([128, C], F32, name="pr_ps", tag="acc")
                for h in range(H):
                    _mm(nc, pr_ps, stacks[h][:, tci * 128:(tci + 1) * 128],
                        pT[h], start=(h == 0), stop=False)
                _mm(nc, pr_ps, ones[0:1, 0:128], projb,
                    start=False, stop=True)
                nc.vector.tensor_add(out_sb[:, tci, :], pr_ps, xT_sb[:, tci, :])
                nc.sync.dma_start(
                    out=yT_d[tci * 128:(tci + 1) * 128, :], in_=out_sb[:, tci, :]
                )

    # Legalize for this walrus: at most 1 sync wait per instruction.
    import bass_rust as _bass_rust
    _bass_rust.move_matmul_waits_to_ldweights(nc.m)
    _bass_rust.generate_event_semaphores(nc)
    return nc


def host_prep(inputs):
    """Per-core input dicts (pure slicing / transpose / permutation)."""
    x = np.ascontiguousarray(np.asarray(inputs["x"], np.float32).reshape(2, C, T))
    qkv_w = np.asarray(inputs["qkv_w"], np.float32)
    proj_w = np.asarray(inputs["proj_w"], np.float32)
    norm_w = np.ascontiguousarray(np.asarray(inputs["norm_w"], np.float32))
    norm_b = np.ascontiguousarray(np.asarray(inputs["norm_b"], np.float32))
    proj_b = np.ascontiguousarray(np.asarray(inputs["proj_b"], np.float32))

    q_idx = np.concatenate([np.arange(h * 192, h * 192 + 64) for h in range(H)])
    wqT = np.ascontiguousarray(qkv_w[q_idx].T)
    wkT = np.ascontiguousarray(qkv_w[q_idx + 64].T)
    wvT = np.ascontiguousarray(qkv_w[q_idx + 128].T)
    pT = np.ascontiguousarray(proj_w.T.reshape(4, 64, C))

    sel = np.zeros((128, 16), np.float32)
    sel[np.arange(128), np.arange(128) // 8] = 1.0 / 8.0
    expand = np.zeros((16, 128), np.float32)
    expand[np.arange(128) // 8, np.arange(128)] = 1.0

    shared = {
        "wqT": wqT, "wkT": wkT, "wvT": wvT, "pT": pT,
        "normw": np.ascontiguousarray(norm_w.reshape(2, 128, 1)),
        "normb": np.ascontiguousarray(norm_b.reshape(2, 128, 1)),
        "projb": np.ascontiguousarray(proj_b.reshape(1, C)),
        "sel": sel, "expand": expand,
        "ones": np.ones((128, 128), np.float32),
    }
    in_maps = []
    for core in range(8):
        b, i = core // 4, core % 4
        t0 = i * TS
        m = dict(shared)
        m["x"] = np.ascontiguousarray(np.roll(x[b], -t0, axis=1))
        m["xT"] = np.ascontiguousarray(x[b, :, t0:t0 + TS].T)
        in_maps.append(m)
    return in_maps


def gather(core_outs):
    y = np.empty((2, C, T), np.float32)
    for core in range(8):
        b, i = core // 4, core % 4
        y[b, :, i * TS:(i + 1) * TS] = core_outs[core].T
    return y.reshape(2, C, 16, 16, 16)


_NC = None


def _get_nc():
    global _NC
    if _NC is None:
        _NC = build_nc()
    return _NC


def run(inputs, trace=False, trace_cores=None):
    nc = _get_nc()
    in_maps = host_prep(inputs)
    res = run_bass_kernel_spmd(
        nc, in_maps, list(range(8)), trace=trace, trace_cores=trace_cores
    )
    out = gather([res.results[c]["yT"] for c in range(8)])
    return out, res


def kernel(**inputs) -> np.ndarray:
    out, _ = run(inputs)
    return out


# revision 11
# speedup vs baseline: 1.2810x; 1.2810x over previous
"""Trainium2 Bass kernel for nn_AttentionBlock_15693810500077.

GroupNorm(32 groups) -> 1x1 qkv conv -> 4-head attention (T=4096) ->
1x1 proj -> residual, for x [2, 256, 16, 16, 16] fp32.

Sharding: 8 cores = (batch b in {0,1}) x (t-slice i in {0..3}, TS=1024).
Each core computes the full attention rows for its t-slice of its batch,
for all 4 heads, plus the projection and residual -> y^T slab [1024, 256].
The host rotates each core's x copy (np.roll over T) so the core's t-slice
always sits at columns 0:1024 -> one static SPMD program for all cores
(softmax over keys is permutation invariant).

Self-contained: hardcodes all shapes; only needs numpy + the concourse
(Bass) runtime available in the environment.
"""
import os

import numpy as np

os.environ.setdefault("JAX_COMPILATION_CACHE_DIR", "/tmp/jaxcache")

import concourse.bass as bass
import concourse.bacc as bacc
import concourse.tile as tile
from concourse import mybir
from concourse.vector_clock import ScopedClock
from concourse.bass_utils import run_bass_kernel_spmd

F32 = mybir.dt.float32
F32R = mybir.dt.float32r
BF16 = mybir.dt.bfloat16
AF = mybir.ActivationFunctionType
ALU = mybir.AluOpType

H = 4
C = 256
T = 4096
TS = 1024
EPS = 1e-5
SCALE2 = 0.125           # (1/sqrt(sqrt(64)))^2
NCHUNKS = T // 128       # 32 key chunks of 128


class _SplitDrainTileContext(tile.TileContext):
    """Walrus in this env rejects >1 sync wait on an SP Drain; split the
    kernel-tail drain's waits across a chain of single-wait drains."""

    def _drain_and_barrier(self, tick_clock, wait_clock):
        drain_inst = self.nc.sync.drain()
        wait_clock.add_sem_waits(
            drain_inst.ins, ScopedClock({None: tick_clock.global_clock})
        )
        si = drain_inst.ins.sync_info
        waits = list(si.on_wait or []) if si is not None else []
        if len(waits) > 1:
            si.on_wait = waits[:1]
            for j in range(1, len(waits)):
                d2 = self.nc.sync.drain()
                d2.ins.sync_info = mybir.SyncInfo(
                    on_wait=waits[j : j + 1], on_update=[]
                )
        self.nc.all_engine_barrier()
        assert self.sems is not None
        popped = self.nc._tile_sem_poison_stack.pop()
        assert popped is self._sem_poison
        self.nc.clear_and_free_semaphores(list(self.sems.allocated().values()))
        self.nc.all_engine_barrier()


def _mm(nc, out, lhsT, rhs, start=True, stop=True, r=True):
    """matmul with fp32r bitcast and N<=512 chunking along the free dim."""
    n = rhs.free_size()
    lt = lhsT.bitcast(F32R) if r else lhsT
    for n0 in range(0, n, 512):
        n1 = min(n0 + 512, n)
        rh = rhs[:, n0:n1]
        nc.tensor.matmul(
            out[:, n0:n1],
            lt,
            rh.bitcast(F32R) if r else rh,
            start=start,
            stop=stop,
        )


def build_nc():
    nc = bass.Bass()

    x_d = nc.dram_tensor("x", [C, T], F32, kind="ExternalInput")
    xT_d = nc.dram_tensor("xT", [TS, C], F32, kind="ExternalInput")
    wqT_d = nc.dram_tensor("wqT", [C, C], F32R, kind="ExternalInput")
    wkT_d = nc.dram_tensor("wkT", [C, C], F32R, kind="ExternalInput")
    wvT_d = nc.dram_tensor("wvT", [C, C], F32R, kind="ExternalInput")
    pT_d = nc.dram_tensor("pT", [4, 64, C], F32R, kind="ExternalInput")
    normw_d = nc.dram_tensor("normw", [2, 128, 1], F32, kind="ExternalInput")
    normb_d = nc.dram_tensor("normb", [2, 128, 1], F32, kind="ExternalInput")
    projb_d = nc.dram_tensor("projb", [1, C], F32R, kind="ExternalInput")
    sel_d = nc.dram_tensor("sel", [128, 16], F32, kind="ExternalInput")
    exp_d = nc.dram_tensor("expand", [16, 128], F32, kind="ExternalInput")
    ones_d = nc.dram_tensor("ones", [128, 128], F32R, kind="ExternalInput")
    onesb_d = nc.dram_tensor("onesb", [128, 128], BF16, kind="ExternalInput")
    yT_d = nc.dram_tensor("yT", [TS, C], F32, kind="ExternalOutput")

    import contextlib

    with tile.TileContext(nc) as tc:
        with (
            tc.tile_pool(name="consts", bufs=1) as consts,
            tc.tile_pool(name="gnp", bufs=2) as gnp,
            tc.tile_pool(name="kqv", bufs=1) as kqv,
            tc.tile_pool(name="psA", bufs=2, space="PSUM") as psA,
            tc.tile_pool(name="psB", bufs=2, space="PSUM") as psB,
            contextlib.ExitStack() as late,
        ):
            # ---- constant / weight loads ----
            wq = [consts.tile([128, C], F32R, name=f"wq{i}") for i in range(2)]
            wk = [consts.tile([128, C], F32R, name=f"wk{i}") for i in range(2)]
            wv = [consts.tile([128, C], F32R, name=f"wv{i}") for i in range(2)]
            for i in range(2):
                nc.sync.dma_start(out=wq[i], in_=wqT_d[i * 128:(i + 1) * 128, :])
                nc.sync.dma_start(out=wk[i], in_=wkT_d[i * 128:(i + 1) * 128, :])
                nc.sync.dma_start(out=wv[i], in_=wvT_d[i * 128:(i + 1) * 128, :])
            pT = [consts.tile([64, C], F32R, name=f"pT{h}") for h in range(H)]
            for h in range(H):
                nc.sync.dma_start(out=pT[h], in_=pT_d[h])
            normw = [consts.tile([128, 1], F32, name=f"nw{i}") for i in range(2)]
            normb = [consts.tile([128, 1], F32, name=f"nb{i}") for i in range(2)]
            for i in range(2):
                nc.sync.dma_start(out=normw[i], in_=normw_d[i])
                nc.sync.dma_start(out=normb[i], in_=normb_d[i])
            projb = consts.tile([1, C], F32R, name="projb")
            nc.sync.dma_start(out=projb, in_=projb_d[:])
            sel = consts.tile([128, 16], F32, name="sel")
            nc.sync.dma_start(out=sel, in_=sel_d[:])
            expand = consts.tile([16, 128], F32, name="expand")
            nc.sync.dma_start(out=expand, in_=exp_d[:])
            xT_sb = consts.tile([128, 8, C], F32, name="xT_sb")
            nc.sync.dma_start(
                out=xT_sb, in_=xT_d.rearrange("(a p) o -> p a o", p=128)
            )
            ones = consts.tile([128, 128], F32R, name="ones")
            nc.sync.dma_start(out=ones, in_=ones_d[:])

            # ---- load x, GroupNorm -> xn ----
            xn = [kqv.tile([128, T], F32R, name=f"xn{i}") for i in range(2)]
            with tc.tile_pool(name="xp", bufs=1) as xp:
                xt = [xp.tile([128, T], F32, name=f"x{i}") for i in range(2)]
                for i in range(2):
                    nc.sync.dma_start(out=xt[i], in_=x_d[i * 128:(i + 1) * 128, :])
                for i in range(2):
                    xv = xt[i].rearrange("p (a f) -> p a f", f=512)
                    stats = gnp.tile([128, 8, 6], F32, name="stats", tag="stats")
                    for j in range(8):
                        nc.vector.bn_stats(out=stats[:, j, :], in_=xv[:, j, :])
                    mv = gnp.tile([128, 2], F32, name="mv", tag="mv")
                    nc.vector.bn_aggr(out=mv, in_=stats)
                    # exsq = var + mean^2
                    msq = gnp.tile([128, 1], F32, name="msq", tag="msq")
                    nc.vector.tensor_mul(msq, mv[:, 0:1], mv[:, 0:1])
                    exsq = gnp.tile([128, 1], F32, name="exsq", tag="exsq")
                    nc.vector.tensor_add(exsq, msq, mv[:, 1:2])
                    # group stats via selector matmuls (plain fp32, tiny)
                    gm_ps = psB.tile([16, 1], F32, name="gm_ps", tag="acc")
                    nc.tensor.matmul(gm_ps, sel, mv[:, 0:1], start=True, stop=True)
                    gx_ps = psB.tile([16, 1], F32, name="gx_ps", tag="acc")
                    nc.tensor.matmul(gx_ps, sel, exsq, start=True, stop=True)
                    gm_sb = gnp.tile([16, 1], F32, name="gm_sb", tag="gm_sb")
                    nc.vector.tensor_copy(gm_sb, gm_ps)
                    gmsq = gnp.tile([16, 1], F32, name="gmsq", tag="gmsq")
                    nc.vector.tensor_mul(gmsq, gm_sb, gm_sb)
                    gvar = gnp.tile([16, 1], F32, name="gvar", tag="gvar")
                    nc.vector.scalar_tensor_tensor(
                        gvar, gx_ps, EPS, gmsq, op0=ALU.add, op1=ALU.subtract
                    )
                    # rstd = exp(-0.5 * ln(var + eps))
                    lnv = gnp.tile([16, 1], F32, name="lnv", tag="lnv")
                    nc.scalar.activation(lnv, gvar, AF.Ln)
                    rstd = gnp.tile([16, 1], F32, name="rstd", tag="rstd")
                    nc.scalar.activation(rstd, lnv, AF.Exp, scale=-0.5)
                    # expand to channels
                    me_ps = psB.tile([128, 1], F32, name="me_ps", tag="acc")
                    nc.tensor.matmul(me_ps, expand, gm_sb, start=True, stop=True)
                    re_ps = psB.tile([128, 1], F32, name="re_ps", tag="acc")
                    nc.tensor.matmul(re_ps, expand, rstd, start=True, stop=True)
                    a_sb = gnp.tile([128, 1], F32, name="a_sb", tag="a_sb")
                    nc.vector.tensor_mul(a_sb, re_ps, normw[i])
                    t2 = gnp.tile([128, 1], F32, name="t2", tag="t2")
                    nc.vector.tensor_mul(t2, me_ps, a_sb)
                    b_sb = gnp.tile([128, 1], F32, name="b_sb", tag="b_sb")
                    nc.vector.tensor_sub(b_sb, normb[i], t2)
                    nc.vector.tensor_scalar(
                        out=xn[i], in0=xt[i], scalar1=a_sb, scalar2=b_sb,
                        op0=ALU.mult, op1=ALU.add,
                    )

            # ---- late pools (opened after the x pool is released) ----
            ppool = late.enter_context(tc.tile_pool(name="ppool", bufs=3))
            rsp = late.enter_context(tc.tile_pool(name="rsp", bufs=2))
            stk = late.enter_context(tc.tile_pool(name="stk", bufs=1))
            outp = late.enter_context(tc.tile_pool(name="outp", bufs=1))

            # ---- qkv ----
            q_sb = [kqv.tile([128, TS], BF16, name=f"q{o}") for o in range(2)]
            k_sb = [kqv.tile([128, T], BF16, name=f"k{o}") for o in range(2)]
            vTa = kqv.tile([128, H, NCHUNKS, 65], BF16, name="vTa")
            nc.sync.dma_start(
                out=vTa[:, :, :, 64:65],
                in_=onesb_d.rearrange("p (a b one) -> p a b one", a=H, one=1),
            )
            for o in range(2):
                q_ps = psA.tile([128, TS], F32, name="q_ps", tag="big")
                for cc in range(2):
                    _mm(nc, q_ps, wq[cc][:, o * 128:(o + 1) * 128],
                        xn[cc][:, 0:TS], start=(cc == 0), stop=(cc == 1))
                nc.vector.tensor_copy(q_sb[o], q_ps)
            for o in range(2):
                for nk in range(8):
                    k_ps = psA.tile([128, 512], F32, name="k_ps", tag="big")
                    for cc in range(2):
                        _mm(nc, k_ps, wk[cc][:, o * 128:(o + 1) * 128],
                            xn[cc][:, nk * 512:(nk + 1) * 512],
                            start=(cc == 0), stop=(cc == 1))
                    nc.vector.tensor_copy(k_sb[o][:, nk * 512:(nk + 1) * 512], k_ps)
            for tci in range(NCHUNKS):
                vt_ps = psA.tile([128, C], F32, name="vt_ps", tag="big")
                for cc in range(2):
                    _mm(nc, vt_ps, xn[cc][:, tci * 128:(tci + 1) * 128],
                        wv[cc], start=(cc == 0), stop=(cc == 1))
                nc.vector.tensor_copy(
                    vTa[:, :, tci, 0:64],
                    vt_ps.rearrange("p (h c) -> p h c", h=H),
                )

            # ---- attention (head pairs share k/q tiles; S^T layout) ----
            stacks = {}
            for pair in ((0, 1), (2, 3)):
                pv_ps = {}
                for h in pair:
                    pv_ps[h] = psB.tile([65, TS], F32, name=f"pv{h}", tag="acc")
                for sc in range(NCHUNKS):
                    p_t = {}
                    for h in pair:
                        kt = k_sb[h // 2]
                        qt = q_sb[h // 2]
                        lo = (h % 2) * 64
                        qk_ps = psA.tile([128, TS], F32, name="qk_ps", tag="big")
                        _mm(nc, qk_ps,
                            kt[lo:lo + 64, sc * 128:(sc + 1) * 128],
                            qt[lo:lo + 64, :], r=False)
                        p_t[h] = ppool.tile([128, TS], BF16, name="p_t", tag="p")
                        nc.scalar.activation(p_t[h], qk_ps, AF.Exp, scale=SCALE2)
                    for h in pair:
                        _mm(nc, pv_ps[h], vTa[:, h, sc, :], p_t[h],
                            start=(sc == 0), stop=(sc == NCHUNKS - 1), r=False)
                # normalize: stack_h = out2 / rowsum
                for h in pair:
                    rs_sb = rsp.tile([65, TS], F32R, name="rs_sb", tag="rs")
                    nc.scalar.copy(rs_sb[64:65, :], pv_ps[h][64:65, :])
                    bc_ps = psA.tile([64, TS], F32, name="bc_ps", tag="big")
                    _mm(nc, bc_ps, ones[64:65, 0:64], rs_sb[64:65, :])
                    recip = rsp.tile([64, TS], F32, name="recip", tag="recip")
                    nc.vector.reciprocal(recip, bc_ps)
                    stack = stk.tile([64, TS], F32R, name=f"stack{h}",
                                     tag=f"stack{h}")
                    nc.vector.tensor_mul(stack, pv_ps[h][0:64, :], recip)
                    stacks[h] = stack

            # ---- proj + bias + residual ----
            out_sb = outp.tile([128, 8, C], F32, name="out_sb")
            for tci in range(8):
                pr_ps = psB.tile([128, C], F32, name="pr_ps", tag="acc")
                for h in range(H):
                    _mm(nc, pr_ps, stacks[h][:, tci * 128:(tci + 1) * 128],
                        pT[h], start=(h == 0), stop=False)
                _mm(nc, pr_ps, ones[0:1, 0:128], projb,
                    start=False, stop=True)
                nc.vector.tensor_add(out_sb[:, tci, :], pr_ps, xT_sb[:, tci, :])
                nc.sync.dma_start(
                    out=yT_d[tci * 128:(tci + 1) * 128, :], in_=out_sb[:, tci, :]
                )

    # Legalize for this walrus: at most 1 sync wait per instruction.
    import bass_rust as _bass_rust
    _bass_rust.move_matmul_waits_to_ldweights(nc.m)
    _bass_rust.generate_event_semaphores(nc)
    return nc


def host_prep(inputs):
    """Per-core input dicts (pure slicing / transpose / permutation)."""
    x = np.ascontiguousarray(np.asarray(inputs["x"], np.float32).reshape(2, C, T))
    qkv_w = np.asarray(inputs["qkv_w"], np.float32)
    proj_w = np.asarray(inputs["proj_w"], np.float32)
    norm_w = np.ascontiguousarray(np.asarray(inputs["norm_w"], np.float32))
    norm_b = np.ascontiguousarray(np.asarray(inputs["norm_b"], np.float32))
    proj_b = np.ascontiguousarray(np.asarray(inputs["proj_b"], np.float32))

    q_idx = np.concatenate([np.arange(h * 192, h * 192 + 64) for h in range(H)])
    wqT = np.ascontiguousarray(qkv_w[q_idx].T)
    wkT = np.ascontiguousarray(qkv_w[q_idx + 64].T)
    wvT = np.ascontiguousarray(qkv_w[q_idx + 128].T)
    pT = np.ascontiguousarray(proj_w.T.reshape(4, 64, C))

    sel = np.zeros((128, 16), np.float32)
    sel[np.arange(128), np.arange(128) // 8] = 1.0 / 8.0
    expand = np.zeros((16, 128), np.float32)
    expand[np.arange(128) // 8, np.arange(128)] = 1.0

    shared = {
        "wqT": wqT, "wkT": wkT, "wvT": wvT, "pT": pT,
        "normw": np.ascontiguousarray(norm_w.reshape(2, 128, 1)),
        "normb": np.ascontiguousarray(norm_b.reshape(2, 128, 1)),
        "projb": np.ascontiguousarray(proj_b.reshape(1, C)),
        "sel": sel, "expand": expand,
        "ones": np.ones((128, 128), np.float32),
        "onesb": np.ones((128, 128), np.float32).astype(
            __import__("ml_dtypes").bfloat16),
    }
    in_maps = []
    for core in range(8):
        b, i = core // 4, core % 4
        t0 = i * TS
        m = dict(shared)
        m["x"] = np.ascontiguousarray(np.roll(x[b], -t0, axis=1))
        m["xT"] = np.ascontiguousarray(x[b, :, t0:t0 + TS].T)
        in_maps.append(m)
    return in_maps


def gather(core_outs):
    y = np.empty((2, C, T), np.float32)
    for core in range(8):
        b, i = core // 4, core % 4
        y[b, :, i * TS:(i + 1) * TS] = core_outs[core].T
    return y.reshape(2, C, 16, 16, 16)


_NC = None


def _get_nc():
    global _NC
    if _NC is None:
        _NC = build_nc()
    return _NC


def run(inputs, trace=False, trace_cores=None):
    nc = _get_nc()
    in_maps = host_prep(inputs)
    res = run_bass_kernel_spmd(
        nc, in_maps, list(range(8)), trace=trace, trace_cores=trace_cores
    )
    out = gather([res.results[c]["yT"] for c in range(8)])
    return out, res


def kernel(**inputs) -> np.ndarray:
    out, _ = run(inputs)
    return out


# revision 15
# speedup vs baseline: 1.2839x; 1.0023x over previous
"""Trainium2 Bass kernel for nn_AttentionBlock_15693810500077.

GroupNorm(32 groups) -> 1x1 qkv conv -> 4-head attention (T=4096) ->
1x1 proj -> residual, for x [2, 256, 16, 16, 16] fp32.

Sharding: 8 cores = (batch b in {0,1}) x (t-slice i in {0..3}, TS=1024).
Each core computes the full attention rows for its t-slice of its batch,
for all 4 heads, plus the projection and residual -> y^T slab [1024, 256].
The host rotates each core's x copy (np.roll over T) so the core's t-slice
always sits at columns 0:1024 -> one static SPMD program for all cores
(softmax over keys is permutation invariant).

Self-contained: hardcodes all shapes; only needs numpy + the concourse
(Bass) runtime available in the environment.
"""
import os

import numpy as np

os.environ.setdefault("JAX_COMPILATION_CACHE_DIR", "/tmp/jaxcache")

import concourse.bass as bass
import concourse.tile as tile
from concourse import mybir
from concourse.bass_utils import run_bass_kernel_spmd

F32 = mybir.dt.float32
F32R = mybir.dt.float32r
BF16 = mybir.dt.bfloat16
AF = mybir.ActivationFunctionType
ALU = mybir.AluOpType

H = 4
C = 256
T = 4096
TS = 1024
EPS = 1e-5
SCALE2 = 0.125           # (1/sqrt(sqrt(64)))^2
NCHUNKS = T // 128       # 32 key chunks of 128


def _mm(nc, out, lhsT, rhs, start=True, stop=True, r=True):
    """matmul with fp32r bitcast and N<=512 chunking along the free dim."""
    n = rhs.free_size()
    lt = lhsT.bitcast(F32R) if r else lhsT
    for n0 in range(0, n, 512):
        n1 = min(n0 + 512, n)
        rh = rhs[:, n0:n1]
        nc.tensor.matmul(
            out[:, n0:n1],
            lt,
            rh.bitcast(F32R) if r else rh,
            start=start,
            stop=stop,
        )


def build_nc():
    nc = bass.Bass()

    x_d = nc.dram_tensor("x", [C, T], F32, kind="ExternalInput")
    xT_d = nc.dram_tensor("xT", [TS, C], F32, kind="ExternalInput")
    wqT_d = nc.dram_tensor("wqT", [C, C], F32R, kind="ExternalInput")
    wkT_d = nc.dram_tensor("wkT", [C, C], F32R, kind="ExternalInput")
    wvT_d = nc.dram_tensor("wvT", [C, C], F32R, kind="ExternalInput")
    pT_d = nc.dram_tensor("pT", [4, 64, C], F32R, kind="ExternalInput")
    normw_d = nc.dram_tensor("normw", [2, 128, 1], F32, kind="ExternalInput")
    normb_d = nc.dram_tensor("normb", [2, 128, 1], F32, kind="ExternalInput")
    projb_d = nc.dram_tensor("projb", [1, C], F32R, kind="ExternalInput")
    sel_d = nc.dram_tensor("sel", [128, 16], F32, kind="ExternalInput")
    exp_d = nc.dram_tensor("expand", [16, 128], F32, kind="ExternalInput")
    ones_d = nc.dram_tensor("ones", [128, 128], F32R, kind="ExternalInput")
    onesb_d = nc.dram_tensor("onesb", [128, 128], BF16, kind="ExternalInput")
    yT_d = nc.dram_tensor("yT", [TS, C], F32, kind="ExternalOutput")

    import contextlib

    with tile.TileContext(nc) as tc:
        with (
            tc.tile_pool(name="consts", bufs=1) as consts,
            tc.tile_pool(name="gnp", bufs=2) as gnp,
            tc.tile_pool(name="kqv", bufs=1) as kqv,
            tc.tile_pool(name="psA", bufs=2, space="PSUM") as psA,
            tc.tile_pool(name="psB", bufs=2, space="PSUM") as psB,
            contextlib.ExitStack() as late,
        ):
            # ---- constant / weight loads ----
            wq = [consts.tile([128, C], F32R, name=f"wq{i}") for i in range(2)]
            wk = [consts.tile([128, C], F32R, name=f"wk{i}") for i in range(2)]
            wv = [consts.tile([128, C], F32R, name=f"wv{i}") for i in range(2)]
            for i in range(2):
                nc.sync.dma_start(out=wq[i], in_=wqT_d[i * 128:(i + 1) * 128, :])
                nc.sync.dma_start(out=wk[i], in_=wkT_d[i * 128:(i + 1) * 128, :])
                nc.sync.dma_start(out=wv[i], in_=wvT_d[i * 128:(i + 1) * 128, :])
            pT = [consts.tile([64, C], F32R, name=f"pT{h}") for h in range(H)]
            for h in range(H):
                nc.sync.dma_start(out=pT[h], in_=pT_d[h])
            normw = [consts.tile([128, 1], F32, name=f"nw{i}") for i in range(2)]
            normb = [consts.tile([128, 1], F32, name=f"nb{i}") for i in range(2)]
            for i in range(2):
                nc.sync.dma_start(out=normw[i], in_=normw_d[i])
                nc.sync.dma_start(out=normb[i], in_=normb_d[i])
            projb = consts.tile([1, C], F32R, name="projb")
            nc.sync.dma_start(out=projb, in_=projb_d[:])
            sel = consts.tile([128, 16], F32, name="sel")
            nc.sync.dma_start(out=sel, in_=sel_d[:])
            expand = consts.tile([16, 128], F32, name="expand")
            nc.sync.dma_start(out=expand, in_=exp_d[:])
            xT_sb = consts.tile([128, 8, C], F32, name="xT_sb")
            nc.sync.dma_start(
                out=xT_sb, in_=xT_d.rearrange("(a p) o -> p a o", p=128)
            )
            ones = consts.tile([128, 128], F32R, name="ones")
            nc.sync.dma_start(out=ones, in_=ones_d[:])

            # ---- load x, GroupNorm -> xn ----
            xn = [kqv.tile([128, T], F32R, name=f"xn{i}") for i in range(2)]
            with tc.tile_pool(name="xp", bufs=1) as xp:
                xt = [xp.tile([128, T], F32, name=f"x{i}") for i in range(2)]
                for i in range(2):
                    nc.sync.dma_start(out=xt[i], in_=x_d[i * 128:(i + 1) * 128, :])
                for i in range(2):
                    xv = xt[i].rearrange("p (a f) -> p a f", f=512)
                    stats = gnp.tile([128, 8, 6], F32, name="stats", tag="stats")
                    for j in range(8):
                        nc.vector.bn_stats(out=stats[:, j, :], in_=xv[:, j, :])
                    mv = gnp.tile([128, 2], F32, name="mv", tag="mv")
                    nc.vector.bn_aggr(out=mv, in_=stats)
                    # exsq = var + mean^2
                    msq = gnp.tile([128, 1], F32, name="msq", tag="msq")
                    nc.vector.tensor_mul(msq, mv[:, 0:1], mv[:, 0:1])
                    exsq = gnp.tile([128, 1], F32, name="exsq", tag="exsq")
                    nc.vector.tensor_add(exsq, msq, mv[:, 1:2])
                    # group stats via selector matmuls (plain fp32, tiny)
                    gm_ps = psB.tile([16, 1], F32, name="gm_ps", tag="acc")
                    nc.tensor.matmul(gm_ps, sel, mv[:, 0:1], start=True, stop=True)
                    gx_ps = psB.tile([16, 1], F32, name="gx_ps", tag="acc")
                    nc.tensor.matmul(gx_ps, sel, exsq, start=True, stop=True)
                    gm_sb = gnp.tile([16, 1], F32, name="gm_sb", tag="gm_sb")
                    nc.vector.tensor_copy(gm_sb, gm_ps)
                    gmsq = gnp.tile([16, 1], F32, name="gmsq", tag="gmsq")
                    nc.vector.tensor_mul(gmsq, gm_sb, gm_sb)
                    gvar = gnp.tile([16, 1], F32, name="gvar", tag="gvar")
                    nc.vector.scalar_tensor_tensor(
                        gvar, gx_ps, EPS, gmsq, op0=ALU.add, op1=ALU.subtract
                    )
                    # rstd = exp(-0.5 * ln(var + eps))
                    lnv = gnp.tile([16, 1], F32, name="lnv", tag="lnv")
                    nc.scalar.activation(lnv, gvar, AF.Ln)
                    rstd = gnp.tile([16, 1], F32, name="rstd", tag="rstd")
                    nc.scalar.activation(rstd, lnv, AF.Exp, scale=-0.5)
                    # expand to channels
                    me_ps = psB.tile([128, 1], F32, name="me_ps", tag="acc")
                    nc.tensor.matmul(me_ps, expand, gm_sb, start=True, stop=True)
                    re_ps = psB.tile([128, 1], F32, name="re_ps", tag="acc")
                    nc.tensor.matmul(re_ps, expand, rstd, start=True, stop=True)
                    a_sb = gnp.tile([128, 1], F32, name="a_sb", tag="a_sb")
                    nc.vector.tensor_mul(a_sb, re_ps, normw[i])
                    t2 = gnp.tile([128, 1], F32, name="t2", tag="t2")
                    nc.vector.tensor_mul(t2, me_ps, a_sb)
                    b_sb = gnp.tile([128, 1], F32, name="b_sb", tag="b_sb")
                    nc.vector.tensor_sub(b_sb, normb[i], t2)
                    nc.vector.tensor_scalar(
                        out=xn[i], in0=xt[i], scalar1=a_sb, scalar2=b_sb,
                        op0=ALU.mult, op1=ALU.add,
                    )

            # ---- late pools (opened after the x pool is released) ----
            ppool = late.enter_context(tc.tile_pool(name="ppool", bufs=3))
            rsp = late.enter_context(tc.tile_pool(name="rsp", bufs=2))
            stk = late.enter_context(tc.tile_pool(name="stk", bufs=1))
            outp = late.enter_context(tc.tile_pool(name="outp", bufs=1))

            # ---- qkv ----
            q_sb = [kqv.tile([128, TS], BF16, name=f"q{o}") for o in range(2)]
            k_sb = [kqv.tile([128, T], BF16, name=f"k{o}") for o in range(2)]
            vTa = kqv.tile([128, H, NCHUNKS, 65], BF16, name="vTa")
            nc.sync.dma_start(
                out=vTa[:, :, :, 64:65],
                in_=onesb_d.rearrange("p (a b one) -> p a b one", a=H, one=1),
            )
            for o in range(2):
                q_ps = psA.tile([128, TS], F32, name="q_ps", tag="big")
                for cc in range(2):
                    _mm(nc, q_ps, wq[cc][:, o * 128:(o + 1) * 128],
                        xn[cc][:, 0:TS], start=(cc == 0), stop=(cc == 1))
                nc.vector.tensor_copy(q_sb[o], q_ps)
            for o in range(2):
                for nk in range(8):
                    k_ps = psA.tile([128, 512], F32, name="k_ps", tag="big")
                    for cc in range(2):
                        _mm(nc, k_ps, wk[cc][:, o * 128:(o + 1) * 128],
                            xn[cc][:, nk * 512:(nk + 1) * 512],
                            start=(cc == 0), stop=(cc == 1))
                    nc.vector.tensor_copy(k_sb[o][:, nk * 512:(nk + 1) * 512], k_ps)
            for tci in range(NCHUNKS):
                vt_ps = psA.tile([128, C], F32, name="vt_ps", tag="big")
                for cc in range(2):
                    _mm(nc, vt_ps, xn[cc][:, tci * 128:(tci + 1) * 128],
                        wv[cc], start=(cc == 0), stop=(cc == 1))
                nc.vector.tensor_copy(
                    vTa[:, :, tci, 0:64],
                    vt_ps.rearrange("p (h c) -> p h c", h=H),
                )

            # ---- attention (head pairs share k/q tiles; S^T layout) ----
            stacks = {}
            for pair in ((0, 1), (2, 3)):
                pv_ps = {}
                for h in pair:
                    pv_ps[h] = psB.tile([65, TS], F32, name=f"pv{h}", tag="acc")
                for sc in range(NCHUNKS):
                    p_t = {}
                    for h in pair:
                        kt = k_sb[h // 2]
                        qt = q_sb[h // 2]
                        lo = (h % 2) * 64
                        qk_ps = psA.tile([128, TS], F32, name="qk_ps", tag="big")
                        _mm(nc, qk_ps,
                            kt[lo:lo + 64, sc * 128:(sc + 1) * 128],
                            qt[lo:lo + 64, :], r=False)
                        p_t[h] = ppool.tile([128, TS], BF16, name="p_t", tag="p")
                        nc.scalar.activation(p_t[h], qk_ps, AF.Exp, scale=SCALE2)
                    for h in pair:
                        _mm(nc, pv_ps[h], vTa[:, h, sc, :], p_t[h],
                            start=(sc == 0), stop=(sc == NCHUNKS - 1), r=False)
                # normalize: stack_h = out2 / rowsum
                for h in pair:
                    rs_sb = rsp.tile([65, TS], F32R, name="rs_sb", tag="rs")
                    nc.scalar.copy(rs_sb[64:65, :], pv_ps[h][64:65, :])
                    bc_ps = psA.tile([64, TS], F32, name="bc_ps", tag="big")
                    _mm(nc, bc_ps, ones[64:65, 0:64], rs_sb[64:65, :])
                    recip = rsp.tile([64, TS], F32, name="recip", tag="recip")
                    nc.vector.reciprocal(recip, bc_ps)
                    stack = stk.tile([64, TS], F32R, name=f"stack{h}",
                                     tag=f"stack{h}")
                    nc.vector.tensor_mul(stack, pv_ps[h][0:64, :], recip)
                    stacks[h] = stack

            # ---- proj + bias + residual ----
            out_sb = outp.tile([128, 8, C], F32, name="out_sb")
            for tci in range(8):
                pr_ps = psB.tile([128, C], F32, name="pr_ps", tag="acc")
                for h in range(H):
                    _mm(nc, pr_ps, stacks[h][:, tci * 128:(tci + 1) * 128],
                        pT[h], start=(h == 0), stop=False)
                _mm(nc, pr_ps, ones[0:1, 0:128], projb,
                    start=False, stop=True)
                nc.vector.tensor_add(out_sb[:, tci, :], pr_ps, xT_sb[:, tci, :])
                nc.sync.dma_start(
                    out=yT_d[tci * 128:(tci + 1) * 128, :], in_=out_sb[:, tci, :]
                )

    # Legalize for this walrus: at most 1 sync wait per instruction.
    import bass_rust as _bass_rust
    _bass_rust.move_matmul_waits_to_ldweights(nc.m)
    _bass_rust.generate_event_semaphores(nc)
    return nc


def host_prep(inputs):
    """Per-core input dicts (pure slicing / transpose / permutation)."""
    x = np.ascontiguousarray(np.asarray(inputs["x"], np.float32).reshape(2, C, T))
    qkv_w = np.asarray(inputs["qkv_w"], np.float32)
    proj_w = np.asarray(inputs["proj_w"], np.float32)
    norm_w = np.ascontiguousarray(np.asarray(inputs["norm_w"], np.float32))
    norm_b = np.ascontiguousarray(np.asarray(inputs["norm_b"], np.float32))
    proj_b = np.ascontiguousarray(np.asarray(inputs["proj_b"], np.float32))

    q_idx = np.concatenate([np.arange(h * 192, h * 192 + 64) for h in range(H)])
    wqT = np.ascontiguousarray(qkv_w[q_idx].T)
    wkT = np.ascontiguousarray(qkv_w[q_idx + 64].T)
    wvT = np.ascontiguousarray(qkv_w[q_idx + 128].T)
    pT = np.ascontiguousarray(proj_w.T.reshape(4, 64, C))

    sel = np.zeros((128, 16), np.float32)
    sel[np.arange(128), np.arange(128) // 8] = 1.0 / 8.0
    expand = np.zeros((16, 128), np.float32)
    expand[np.arange(128) // 8, np.arange(128)] = 1.0

    shared = {
        "wqT": wqT, "wkT": wkT, "wvT": wvT, "pT": pT,
        "normw": np.ascontiguousarray(norm_w.reshape(2, 128, 1)),
        "normb": np.ascontiguousarray(norm_b.reshape(2, 128, 1)),
        "projb": np.ascontiguousarray(proj_b.reshape(1, C)),
        "sel": sel, "expand": expand,
        "ones": np.ones((128, 128), np.float32),
        "onesb": np.ones((128, 128), np.float32).astype(
            __import__("ml_dtypes").bfloat16),
    }
    in_maps = []
    for core in range(8):
        b, i = core // 4, core % 4
        t0 = i * TS
        m = dict(shared)
        m["x"] = np.ascontiguousarray(np.roll(x[b], -t0, axis=1))
        m["xT"] = np.ascontiguousarray(x[b, :, t0:t0 + TS].T)
        in_maps.append(m)
    return in_maps


def gather(core_outs):
    y = np.empty((2, C, T), np.float32)
    for core in range(8):
        b, i = core // 4, core % 4
        y[b, :, i * TS:(i + 1) * TS] = core_outs[core].T
    return y.reshape(2, C, 16, 16, 16)


_NC = None


def _get_nc():
    global _NC
    if _NC is None:
        _NC = build_nc()
    return _NC


def run(inputs, trace=False, trace_cores=None):
    nc = _get_nc()
    in_maps = host_prep(inputs)
    res = run_bass_kernel_spmd(
        nc, in_maps, list(range(8)), trace=trace, trace_cores=trace_cores
    )
    out = gather([res.results[c]["yT"] for c in range(8)])
    return out, res


def kernel(**inputs) -> np.ndarray:
    out, _ = run(inputs)
    return out


# revision 16
# speedup vs baseline: 1.3125x; 1.0223x over previous
"""Trainium2 Bass kernel for nn_AttentionBlock_15693810500077.

GroupNorm(32 groups) -> 1x1 qkv conv -> 4-head attention (T=4096) ->
1x1 proj -> residual, for x [2, 256, 16, 16, 16] fp32.

Sharding: 8 cores = (batch b in {0,1}) x (t-slice i in {0..3}, TS=1024).
Each core computes the full attention rows for its t-slice of its batch,
for all 4 heads, plus the projection and residual -> y^T slab [1024, 256].
The host rotates each core's x copy (np.roll over T) so the core's t-slice
always sits at columns 0:1024 -> one static SPMD program for all cores
(softmax over keys is permutation invariant).

Self-contained: hardcodes all shapes; only needs numpy + the concourse
(Bass) runtime available in the environment.
"""
import os

import numpy as np

os.environ.setdefault("JAX_COMPILATION_CACHE_DIR", "/tmp/jaxcache")

import concourse.bass as bass
import concourse.tile as tile
from concourse import mybir
from concourse.bass_utils import run_bass_kernel_spmd

F32 = mybir.dt.float32
F32R = mybir.dt.float32r
BF16 = mybir.dt.bfloat16
AF = mybir.ActivationFunctionType
ALU = mybir.AluOpType

H = 4
C = 256
T = 4096
TS = 1024
EPS = 1e-5
SCALE2 = 0.125           # (1/sqrt(sqrt(64)))^2
NCHUNKS = T // 128       # 32 key chunks of 128


def _mm(nc, out, lhsT, rhs, start=True, stop=True, r=True):
    """matmul with fp32r bitcast and N<=512 chunking along the free dim."""
    n = rhs.free_size()
    lt = lhsT.bitcast(F32R) if r else lhsT
    for n0 in range(0, n, 512):
        n1 = min(n0 + 512, n)
        rh = rhs[:, n0:n1]
        nc.tensor.matmul(
            out[:, n0:n1],
            lt,
            rh.bitcast(F32R) if r else rh,
            start=start,
            stop=stop,
        )


def build_nc():
    nc = bass.Bass()

    x_d = nc.dram_tensor("x", [C, T], F32, kind="ExternalInput")
    xT_d = nc.dram_tensor("xT", [TS, C], F32, kind="ExternalInput")
    wqT_d = nc.dram_tensor("wqT", [C, C], BF16, kind="ExternalInput")
    wkT_d = nc.dram_tensor("wkT", [C, C], BF16, kind="ExternalInput")
    wvT_d = nc.dram_tensor("wvT", [C, C], BF16, kind="ExternalInput")
    pT_d = nc.dram_tensor("pT", [4, 64, C], F32R, kind="ExternalInput")
    normw_d = nc.dram_tensor("normw", [2, 128, 1], F32, kind="ExternalInput")
    normb_d = nc.dram_tensor("normb", [2, 128, 1], F32, kind="ExternalInput")
    projb_d = nc.dram_tensor("projb", [1, C], F32R, kind="ExternalInput")
    sel_d = nc.dram_tensor("sel", [128, 16], F32, kind="ExternalInput")
    exp_d = nc.dram_tensor("expand", [16, 128], F32, kind="ExternalInput")
    ones_d = nc.dram_tensor("ones", [128, 128], F32R, kind="ExternalInput")
    onesb_d = nc.dram_tensor("onesb", [128, 128], BF16, kind="ExternalInput")
    yT_d = nc.dram_tensor("yT", [TS, C], F32, kind="ExternalOutput")

    import contextlib

    with tile.TileContext(nc) as tc:
        with (
            tc.tile_pool(name="consts", bufs=1) as consts,
            tc.tile_pool(name="gnp", bufs=2) as gnp,
            tc.tile_pool(name="kqv", bufs=1) as kqv,
            tc.tile_pool(name="psA", bufs=2, space="PSUM") as psA,
            tc.tile_pool(name="psB", bufs=2, space="PSUM") as psB,
            contextlib.ExitStack() as late,
        ):
            # ---- constant / weight loads ----
            wq = [consts.tile([128, C], BF16, name=f"wq{i}") for i in range(2)]
            wk = [consts.tile([128, C], BF16, name=f"wk{i}") for i in range(2)]
            wv = [consts.tile([128, C], BF16, name=f"wv{i}") for i in range(2)]
            for i in range(2):
                nc.sync.dma_start(out=wq[i], in_=wqT_d[i * 128:(i + 1) * 128, :])
                nc.sync.dma_start(out=wk[i], in_=wkT_d[i * 128:(i + 1) * 128, :])
                nc.sync.dma_start(out=wv[i], in_=wvT_d[i * 128:(i + 1) * 128, :])
            pT = [consts.tile([64, C], F32R, name=f"pT{h}") for h in range(H)]
            for h in range(H):
                nc.sync.dma_start(out=pT[h], in_=pT_d[h])
            normw = [consts.tile([128, 1], F32, name=f"nw{i}") for i in range(2)]
            normb = [consts.tile([128, 1], F32, name=f"nb{i}") for i in range(2)]
            for i in range(2):
                nc.sync.dma_start(out=normw[i], in_=normw_d[i])
                nc.sync.dma_start(out=normb[i], in_=normb_d[i])
            projb = consts.tile([1, C], F32R, name="projb")
            nc.sync.dma_start(out=projb, in_=projb_d[:])
            sel = consts.tile([128, 16], F32, name="sel")
            nc.sync.dma_start(out=sel, in_=sel_d[:])
            expand = consts.tile([16, 128], F32, name="expand")
            nc.sync.dma_start(out=expand, in_=exp_d[:])
            xT_sb = consts.tile([128, 8, C], F32, name="xT_sb")
            nc.sync.dma_start(
                out=xT_sb, in_=xT_d.rearrange("(a p) o -> p a o", p=128)
            )
            ones = consts.tile([128, 128], F32R, name="ones")
            nc.sync.dma_start(out=ones, in_=ones_d[:])

            # ---- load x, GroupNorm -> xn ----
            xn = [kqv.tile([128, T], BF16, name=f"xn{i}") for i in range(2)]
            with tc.tile_pool(name="xp", bufs=1) as xp:
                xt = [xp.tile([128, T], F32, name=f"x{i}") for i in range(2)]
                for i in range(2):
                    for jc in range(4):
                        nc.sync.dma_start(
                            out=xt[i][:, jc * 1024:(jc + 1) * 1024],
                            in_=x_d[i * 128:(i + 1) * 128,
                                    jc * 1024:(jc + 1) * 1024],
                        )
                for i in range(2):
                    xv = xt[i].rearrange("p (a f) -> p a f", f=512)
                    stats = gnp.tile([128, 8, 6], F32, name="stats", tag="stats")
                    for j in range(8):
                        nc.vector.bn_stats(out=stats[:, j, :], in_=xv[:, j, :])
                    mv = gnp.tile([128, 2], F32, name="mv", tag="mv")
                    nc.vector.bn_aggr(out=mv, in_=stats)
                    # exsq = var + mean^2
                    msq = gnp.tile([128, 1], F32, name="msq", tag="msq")
                    nc.vector.tensor_mul(msq, mv[:, 0:1], mv[:, 0:1])
                    exsq = gnp.tile([128, 1], F32, name="exsq", tag="exsq")
                    nc.vector.tensor_add(exsq, msq, mv[:, 1:2])
                    # group stats via selector matmuls (plain fp32, tiny)
                    gm_ps = psB.tile([16, 1], F32, name="gm_ps", tag="acc")
                    nc.tensor.matmul(gm_ps, sel, mv[:, 0:1], start=True, stop=True)
                    gx_ps = psB.tile([16, 1], F32, name="gx_ps", tag="acc")
                    nc.tensor.matmul(gx_ps, sel, exsq, start=True, stop=True)
                    gm_sb = gnp.tile([16, 1], F32, name="gm_sb", tag="gm_sb")
                    nc.vector.tensor_copy(gm_sb, gm_ps)
                    gmsq = gnp.tile([16, 1], F32, name="gmsq", tag="gmsq")
                    nc.vector.tensor_mul(gmsq, gm_sb, gm_sb)
                    gvar = gnp.tile([16, 1], F32, name="gvar", tag="gvar")
                    nc.vector.scalar_tensor_tensor(
                        gvar, gx_ps, EPS, gmsq, op0=ALU.add, op1=ALU.subtract
                    )
                    # rstd = exp(-0.5 * ln(var + eps))
                    lnv = gnp.tile([16, 1], F32, name="lnv", tag="lnv")
                    nc.scalar.activation(lnv, gvar, AF.Ln)
                    rstd = gnp.tile([16, 1], F32, name="rstd", tag="rstd")
                    nc.scalar.activation(rstd, lnv, AF.Exp, scale=-0.5)
                    # expand to channels
                    me_ps = psB.tile([128, 1], F32, name="me_ps", tag="acc")
                    nc.tensor.matmul(me_ps, expand, gm_sb, start=True, stop=True)
                    re_ps = psB.tile([128, 1], F32, name="re_ps", tag="acc")
                    nc.tensor.matmul(re_ps, expand, rstd, start=True, stop=True)
                    a_sb = gnp.tile([128, 1], F32, name="a_sb", tag="a_sb")
                    nc.vector.tensor_mul(a_sb, re_ps, normw[i])
                    t2 = gnp.tile([128, 1], F32, name="t2", tag="t2")
                    nc.vector.tensor_mul(t2, me_ps, a_sb)
                    b_sb = gnp.tile([128, 1], F32, name="b_sb", tag="b_sb")
                    nc.vector.tensor_sub(b_sb, normb[i], t2)
                    nc.vector.tensor_scalar(
                        out=xn[i], in0=xt[i], scalar1=a_sb, scalar2=b_sb,
                        op0=ALU.mult, op1=ALU.add,
                    )

            # ---- late pools (opened after the x pool is released) ----
            ppool = late.enter_context(tc.tile_pool(name="ppool", bufs=3))
            rsp = late.enter_context(tc.tile_pool(name="rsp", bufs=2))
            stk = late.enter_context(tc.tile_pool(name="stk", bufs=1))
            outp = late.enter_context(tc.tile_pool(name="outp", bufs=1))

            # ---- qkv ----
            q_sb = [kqv.tile([128, TS], BF16, name=f"q{o}") for o in range(2)]
            k_sb = [kqv.tile([128, T], BF16, name=f"k{o}") for o in range(2)]
            vTa = kqv.tile([128, H, NCHUNKS, 65], BF16, name="vTa")
            nc.sync.dma_start(
                out=vTa[:, :, :, 64:65],
                in_=onesb_d.rearrange("p (a b one) -> p a b one", a=H, one=1),
            )
            for o in range(2):
                q_ps = psA.tile([128, TS], F32, name="q_ps", tag="big")
                for cc in range(2):
                    _mm(nc, q_ps, wq[cc][:, o * 128:(o + 1) * 128],
                        xn[cc][:, 0:TS], start=(cc == 0), stop=(cc == 1),
                        r=False)
                nc.vector.tensor_copy(q_sb[o], q_ps)
            for o in range(2):
                for nk in range(8):
                    k_ps = psA.tile([128, 512], F32, name="k_ps", tag="big")
                    for cc in range(2):
                        _mm(nc, k_ps, wk[cc][:, o * 128:(o + 1) * 128],
                            xn[cc][:, nk * 512:(nk + 1) * 512],
                            start=(cc == 0), stop=(cc == 1), r=False)
                    nc.vector.tensor_copy(k_sb[o][:, nk * 512:(nk + 1) * 512], k_ps)
            for tci in range(NCHUNKS):
                vt_ps = psA.tile([128, C], F32, name="vt_ps", tag="big")
                for cc in range(2):
                    _mm(nc, vt_ps, xn[cc][:, tci * 128:(tci + 1) * 128],
                        wv[cc], start=(cc == 0), stop=(cc == 1), r=False)
                nc.vector.tensor_copy(
                    vTa[:, :, tci, 0:64],
                    vt_ps.rearrange("p (h c) -> p h c", h=H),
                )

            # ---- attention (head pairs share k/q tiles; S^T layout) ----
            stacks = {}
            for pair in ((0, 1), (2, 3)):
                pv_ps = {}
                for h in pair:
                    pv_ps[h] = psB.tile([65, TS], F32, name=f"pv{h}", tag="acc")
                for sc in range(NCHUNKS):
                    p_t = {}
                    for h in pair:
                        kt = k_sb[h // 2]
                        qt = q_sb[h // 2]
                        lo = (h % 2) * 64
                        qk_ps = psA.tile([128, TS], F32, name="qk_ps", tag="big")
                        _mm(nc, qk_ps,
                            kt[lo:lo + 64, sc * 128:(sc + 1) * 128],
                            qt[lo:lo + 64, :], r=False)
                        p_t[h] = ppool.tile([128, TS], BF16, name="p_t", tag="p")
                        nc.scalar.activation(p_t[h], qk_ps, AF.Exp, scale=SCALE2)
                    for h in pair:
                        _mm(nc, pv_ps[h], vTa[:, h, sc, :], p_t[h],
                            start=(sc == 0), stop=(sc == NCHUNKS - 1), r=False)
                # normalize: stack_h = out2 / rowsum
                for h in pair:
                    rs_sb = rsp.tile([65, TS], F32R, name="rs_sb", tag="rs")
                    nc.scalar.copy(rs_sb[64:65, :], pv_ps[h][64:65, :])
                    bc_ps = psA.tile([64, TS], F32, name="bc_ps", tag="big")
                    _mm(nc, bc_ps, ones[64:65, 0:64], rs_sb[64:65, :])
                    recip = rsp.tile([64, TS], F32, name="recip", tag="recip")
                    nc.vector.reciprocal(recip, bc_ps)
                    stack = stk.tile([64, TS], F32R, name=f"stack{h}",
                                     tag=f"stack{h}")
                    nc.vector.tensor_mul(stack, pv_ps[h][0:64, :], recip)
                    stacks[h] = stack

            # ---- proj + bias + residual ----
            out_sb = outp.tile([128, 8, C], F32, name="out_sb")
            for tci in range(8):
                pr_ps = psB.tile([128, C], F32, name="pr_ps", tag="acc")
                for h in range(H):
                    _mm(nc, pr_ps, stacks[h][:, tci * 128:(tci + 1) * 128],
                        pT[h], start=(h == 0), stop=False)
                _mm(nc, pr_ps, ones[0:1, 0:128], projb,
                    start=False, stop=True)
                nc.vector.tensor_add(out_sb[:, tci, :], pr_ps, xT_sb[:, tci, :])
                nc.sync.dma_start(
                    out=yT_d[tci * 128:(tci + 1) * 128, :], in_=out_sb[:, tci, :]
                )

    # Legalize for this walrus: at most 1 sync wait per instruction.
    import bass_rust as _bass_rust
    _bass_rust.move_matmul_waits_to_ldweights(nc.m)
    _bass_rust.generate_event_semaphores(nc)
    return nc


def host_prep(inputs):
    """Per-core input dicts (pure slicing / transpose / permutation)."""
    x = np.ascontiguousarray(np.asarray(inputs["x"], np.float32).reshape(2, C, T))
    qkv_w = np.asarray(inputs["qkv_w"], np.float32)
    proj_w = np.asarray(inputs["proj_w"], np.float32)
    norm_w = np.ascontiguousarray(np.asarray(inputs["norm_w"], np.float32))
    norm_b = np.ascontiguousarray(np.asarray(inputs["norm_b"], np.float32))
    proj_b = np.ascontiguousarray(np.asarray(inputs["proj_b"], np.float32))

    q_idx = np.concatenate([np.arange(h * 192, h * 192 + 64) for h in range(H)])
    wqT = np.ascontiguousarray(qkv_w[q_idx].T)
    wkT = np.ascontiguousarray(qkv_w[q_idx + 64].T)
    wvT = np.ascontiguousarray(qkv_w[q_idx + 128].T)
    pT = np.ascontiguousarray(proj_w.T.reshape(4, 64, C))

    sel = np.zeros((128, 16), np.float32)
    sel[np.arange(128), np.arange(128) // 8] = 1.0 / 8.0
    expand = np.zeros((16, 128), np.float32)
    expand[np.arange(128) // 8, np.arange(128)] = 1.0

    bf = __import__("ml_dtypes").bfloat16
    shared = {
        "wqT": wqT.astype(bf), "wkT": wkT.astype(bf), "wvT": wvT.astype(bf),
        "pT": pT,
        "normw": np.ascontiguousarray(norm_w.reshape(2, 128, 1)),
        "normb": np.ascontiguousarray(norm_b.reshape(2, 128, 1)),
        "projb": np.ascontiguousarray(proj_b.reshape(1, C)),
        "sel": sel, "expand": expand,
        "ones": np.ones((128, 128), np.float32),
        "onesb": np.ones((128, 128), np.float32).astype(
            __import__("ml_dtypes").bfloat16),
    }
    in_maps = []
    for core in range(8):
        b, i = core // 4, core % 4
        t0 = i * TS
        m = dict(shared)
        m["x"] = np.ascontiguousarray(np.roll(x[b], -t0, axis=1))
        m["xT"] = np.ascontiguousarray(x[b, :, t0:t0 + TS].T)
        in_maps.append(m)
    return in_maps


def gather(core_outs):
    y = np.empty((2, C, T), np.float32)
    for core in range(8):
        b, i = core // 4, core % 4
        y[b, :, i * TS:(i + 1) * TS] = core_outs[core].T
    return y.reshape(2, C, 16, 16, 16)


_NC = None


def _get_nc():
    global _NC
    if _NC is None:
        _NC = build_nc()
    return _NC


def run(inputs, trace=False, trace_cores=None):
    nc = _get_nc()
    in_maps = host_prep(inputs)
    res = run_bass_kernel_spmd(
        nc, in_maps, list(range(8)), trace=trace, trace_cores=trace_cores
    )
    out = gather([res.results[c]["yT"] for c in range(8)])
    return out, res


def kernel(**inputs) -> np.ndarray:
    out, _ = run(inputs)
    return out


# revision 17
# speedup vs baseline: 1.3200x; 1.0057x over previous
"""Trainium2 Bass kernel for nn_AttentionBlock_15693810500077.

GroupNorm(32 groups) -> 1x1 qkv conv -> 4-head attention (T=4096) ->
1x1 proj -> residual, for x [2, 256, 16, 16, 16] fp32.

Sharding: 8 cores = (batch b in {0,1}) x (t-slice i in {0..3}, TS=1024).
Each core computes the full attention rows for its t-slice of its batch,
for all 4 heads, plus the projection and residual -> y^T slab [1024, 256].
The host rotates each core's x copy (np.roll over T) so the core's t-slice
always sits at columns 0:1024 -> one static SPMD program for all cores
(softmax over keys is permutation invariant).

Self-contained: hardcodes all shapes; only needs numpy + the concourse
(Bass) runtime available in the environment.
"""
import os

import numpy as np

os.environ.setdefault("JAX_COMPILATION_CACHE_DIR", "/tmp/jaxcache")

import concourse.bass as bass
import concourse.tile as tile
from concourse import mybir
from concourse.bass_utils import run_bass_kernel_spmd

F32 = mybir.dt.float32
F32R = mybir.dt.float32r
BF16 = mybir.dt.bfloat16
AF = mybir.ActivationFunctionType
ALU = mybir.AluOpType

H = 4
C = 256
T = 4096
TS = 1024
EPS = 1e-5
SCALE2 = 0.125           # (1/sqrt(sqrt(64)))^2
NCHUNKS = T // 128       # 32 key chunks of 128


def _mm(nc, out, lhsT, rhs, start=True, stop=True, r=True):
    """matmul with fp32r bitcast and N<=512 chunking along the free dim."""
    n = rhs.free_size()
    lt = lhsT.bitcast(F32R) if r else lhsT
    for n0 in range(0, n, 512):
        n1 = min(n0 + 512, n)
        rh = rhs[:, n0:n1]
        nc.tensor.matmul(
            out[:, n0:n1],
            lt,
            rh.bitcast(F32R) if r else rh,
            start=start,
            stop=stop,
        )


def build_nc():
    nc = bass.Bass()

    x_d = nc.dram_tensor("x", [C, T], F32, kind="ExternalInput")
    xT_d = nc.dram_tensor("xT", [TS, C], F32, kind="ExternalInput")
    wqT_d = nc.dram_tensor("wqT", [C, C], BF16, kind="ExternalInput")
    wkT_d = nc.dram_tensor("wkT", [C, C], BF16, kind="ExternalInput")
    wvT_d = nc.dram_tensor("wvT", [C, C], BF16, kind="ExternalInput")
    pT_d = nc.dram_tensor("pT", [4, 64, C], BF16, kind="ExternalInput")
    normw_d = nc.dram_tensor("normw", [2, 128, 1], F32, kind="ExternalInput")
    normb_d = nc.dram_tensor("normb", [2, 128, 1], F32, kind="ExternalInput")
    projb_d = nc.dram_tensor("projb", [1, C], F32R, kind="ExternalInput")
    sel_d = nc.dram_tensor("sel", [128, 16], F32, kind="ExternalInput")
    exp_d = nc.dram_tensor("expand", [16, 128], F32, kind="ExternalInput")
    ones_d = nc.dram_tensor("ones", [128, 128], F32R, kind="ExternalInput")
    onesb_d = nc.dram_tensor("onesb", [128, 128], BF16, kind="ExternalInput")
    yT_d = nc.dram_tensor("yT", [TS, C], F32, kind="ExternalOutput")

    import contextlib

    with tile.TileContext(nc) as tc:
        with (
            tc.tile_pool(name="consts", bufs=1) as consts,
            tc.tile_pool(name="gnp", bufs=2) as gnp,
            tc.tile_pool(name="kqv", bufs=1) as kqv,
            tc.tile_pool(name="psA", bufs=2, space="PSUM") as psA,
            tc.tile_pool(name="psB", bufs=2, space="PSUM") as psB,
            contextlib.ExitStack() as late,
        ):
            # ---- constant / weight loads ----
            wq = [consts.tile([128, C], BF16, name=f"wq{i}") for i in range(2)]
            wk = [consts.tile([128, C], BF16, name=f"wk{i}") for i in range(2)]
            wv = [consts.tile([128, C], BF16, name=f"wv{i}") for i in range(2)]
            for i in range(2):
                nc.sync.dma_start(out=wq[i], in_=wqT_d[i * 128:(i + 1) * 128, :])
                nc.sync.dma_start(out=wk[i], in_=wkT_d[i * 128:(i + 1) * 128, :])
                nc.sync.dma_start(out=wv[i], in_=wvT_d[i * 128:(i + 1) * 128, :])
            pT = [consts.tile([64, C], BF16, name=f"pT{h}") for h in range(H)]
            for h in range(H):
                nc.sync.dma_start(out=pT[h], in_=pT_d[h])
            normw = [consts.tile([128, 1], F32, name=f"nw{i}") for i in range(2)]
            normb = [consts.tile([128, 1], F32, name=f"nb{i}") for i in range(2)]
            for i in range(2):
                nc.sync.dma_start(out=normw[i], in_=normw_d[i])
                nc.sync.dma_start(out=normb[i], in_=normb_d[i])
            projb = consts.tile([1, C], F32R, name="projb")
            nc.sync.dma_start(out=projb, in_=projb_d[:])
            sel = consts.tile([128, 16], F32, name="sel")
            nc.sync.dma_start(out=sel, in_=sel_d[:])
            expand = consts.tile([16, 128], F32, name="expand")
            nc.sync.dma_start(out=expand, in_=exp_d[:])
            xT_sb = consts.tile([128, 8, C], F32, name="xT_sb")
            nc.sync.dma_start(
                out=xT_sb, in_=xT_d.rearrange("(a p) o -> p a o", p=128)
            )
            ones = consts.tile([128, 128], F32R, name="ones")
            nc.sync.dma_start(out=ones, in_=ones_d[:])

            # ---- load x, GroupNorm -> xn ----
            xn = [kqv.tile([128, T], BF16, name=f"xn{i}") for i in range(2)]
            with tc.tile_pool(name="xp", bufs=1) as xp:
                xt = [xp.tile([128, T], F32, name=f"x{i}") for i in range(2)]
                for i in range(2):
                    for jc in range(4):
                        nc.sync.dma_start(
                            out=xt[i][:, jc * 1024:(jc + 1) * 1024],
                            in_=x_d[i * 128:(i + 1) * 128,
                                    jc * 1024:(jc + 1) * 1024],
                        )
                for i in range(2):
                    xv = xt[i].rearrange("p (a f) -> p a f", f=512)
                    stats = gnp.tile([128, 8, 6], F32, name="stats", tag="stats")
                    for j in range(8):
                        nc.vector.bn_stats(out=stats[:, j, :], in_=xv[:, j, :])
                    mv = gnp.tile([128, 2], F32, name="mv", tag="mv")
                    nc.vector.bn_aggr(out=mv, in_=stats)
                    # exsq = var + mean^2
                    msq = gnp.tile([128, 1], F32, name="msq", tag="msq")
                    nc.vector.tensor_mul(msq, mv[:, 0:1], mv[:, 0:1])
                    exsq = gnp.tile([128, 1], F32, name="exsq", tag="exsq")
                    nc.vector.tensor_add(exsq, msq, mv[:, 1:2])
                    # group stats via selector matmuls (plain fp32, tiny)
                    gm_ps = psB.tile([16, 1], F32, name="gm_ps", tag="acc")
                    nc.tensor.matmul(gm_ps, sel, mv[:, 0:1], start=True, stop=True)
                    gx_ps = psB.tile([16, 1], F32, name="gx_ps", tag="acc")
                    nc.tensor.matmul(gx_ps, sel, exsq, start=True, stop=True)
                    gm_sb = gnp.tile([16, 1], F32, name="gm_sb", tag="gm_sb")
                    nc.vector.tensor_copy(gm_sb, gm_ps)
                    gmsq = gnp.tile([16, 1], F32, name="gmsq", tag="gmsq")
                    nc.vector.tensor_mul(gmsq, gm_sb, gm_sb)
                    gvar = gnp.tile([16, 1], F32, name="gvar", tag="gvar")
                    nc.vector.scalar_tensor_tensor(
                        gvar, gx_ps, EPS, gmsq, op0=ALU.add, op1=ALU.subtract
                    )
                    # rstd = exp(-0.5 * ln(var + eps))
                    lnv = gnp.tile([16, 1], F32, name="lnv", tag="lnv")
                    nc.scalar.activation(lnv, gvar, AF.Ln)
                    rstd = gnp.tile([16, 1], F32, name="rstd", tag="rstd")
                    nc.scalar.activation(rstd, lnv, AF.Exp, scale=-0.5)
                    # expand to channels
                    me_ps = psB.tile([128, 1], F32, name="me_ps", tag="acc")
                    nc.tensor.matmul(me_ps, expand, gm_sb, start=True, stop=True)
                    re_ps = psB.tile([128, 1], F32, name="re_ps", tag="acc")
                    nc.tensor.matmul(re_ps, expand, rstd, start=True, stop=True)
                    a_sb = gnp.tile([128, 1], F32, name="a_sb", tag="a_sb")
                    nc.vector.tensor_mul(a_sb, re_ps, normw[i])
                    t2 = gnp.tile([128, 1], F32, name="t2", tag="t2")
                    nc.vector.tensor_mul(t2, me_ps, a_sb)
                    b_sb = gnp.tile([128, 1], F32, name="b_sb", tag="b_sb")
                    nc.vector.tensor_sub(b_sb, normb[i], t2)
                    nc.vector.tensor_scalar(
                        out=xn[i], in0=xt[i], scalar1=a_sb, scalar2=b_sb,
                        op0=ALU.mult, op1=ALU.add,
                    )

            # ---- late pools (opened after the x pool is released) ----
            ppool = late.enter_context(tc.tile_pool(name="ppool", bufs=3))
            rsp = late.enter_context(tc.tile_pool(name="rsp", bufs=2))
            stk = late.enter_context(tc.tile_pool(name="stk", bufs=1))
            outp = late.enter_context(tc.tile_pool(name="outp", bufs=1))

            # ---- qkv ----
            q_sb = [kqv.tile([128, TS], BF16, name=f"q{o}") for o in range(2)]
            k_sb = [kqv.tile([128, T], BF16, name=f"k{o}") for o in range(2)]
            vTa = kqv.tile([128, H, NCHUNKS, 65], BF16, name="vTa")
            nc.sync.dma_start(
                out=vTa[:, :, :, 64:65],
                in_=onesb_d.rearrange("p (a b one) -> p a b one", a=H, one=1),
            )
            for o in range(2):
                q_ps = psA.tile([128, TS], F32, name="q_ps", tag="big")
                for cc in range(2):
                    _mm(nc, q_ps, wq[cc][:, o * 128:(o + 1) * 128],
                        xn[cc][:, 0:TS], start=(cc == 0), stop=(cc == 1),
                        r=False)
                nc.vector.tensor_copy(q_sb[o], q_ps)
            for o in range(2):
                for nk in range(8):
                    k_ps = psA.tile([128, 512], F32, name="k_ps", tag="big")
                    for cc in range(2):
                        _mm(nc, k_ps, wk[cc][:, o * 128:(o + 1) * 128],
                            xn[cc][:, nk * 512:(nk + 1) * 512],
                            start=(cc == 0), stop=(cc == 1), r=False)
                    nc.vector.tensor_copy(k_sb[o][:, nk * 512:(nk + 1) * 512], k_ps)
            for tci in range(NCHUNKS):
                vt_ps = psA.tile([128, C], F32, name="vt_ps", tag="big")
                for cc in range(2):
                    _mm(nc, vt_ps, xn[cc][:, tci * 128:(tci + 1) * 128],
                        wv[cc], start=(cc == 0), stop=(cc == 1), r=False)
                nc.vector.tensor_copy(
                    vTa[:, :, tci, 0:64],
                    vt_ps.rearrange("p (h c) -> p h c", h=H),
                )

            # ---- attention (head pairs share k/q tiles; S^T layout) ----
            stacks = {}
            for pair in ((0, 1), (2, 3)):
                pv_ps = {}
                for h in pair:
                    pv_ps[h] = psB.tile([65, TS], F32, name=f"pv{h}", tag="acc")
                for sc in range(NCHUNKS):
                    p_t = {}
                    for h in pair:
                        kt = k_sb[h // 2]
                        qt = q_sb[h // 2]
                        lo = (h % 2) * 64
                        qk_ps = psA.tile([128, TS], F32, name="qk_ps", tag="big")
                        _mm(nc, qk_ps,
                            kt[lo:lo + 64, sc * 128:(sc + 1) * 128],
                            qt[lo:lo + 64, :], r=False)
                        p_t[h] = ppool.tile([128, TS], BF16, name="p_t", tag="p")
                        nc.scalar.activation(p_t[h], qk_ps, AF.Exp, scale=SCALE2)
                    for h in pair:
                        _mm(nc, pv_ps[h], vTa[:, h, sc, :], p_t[h],
                            start=(sc == 0), stop=(sc == NCHUNKS - 1), r=False)
                # normalize: stack_h = out2 / rowsum
                for h in pair:
                    rs_sb = rsp.tile([65, TS], F32R, name="rs_sb", tag="rs")
                    nc.scalar.copy(rs_sb[64:65, :], pv_ps[h][64:65, :])
                    bc_ps = psA.tile([64, TS], F32, name="bc_ps", tag="big")
                    _mm(nc, bc_ps, ones[64:65, 0:64], rs_sb[64:65, :])
                    recip = rsp.tile([64, TS], F32, name="recip", tag="recip")
                    nc.vector.reciprocal(recip, bc_ps)
                    stack = stk.tile([64, TS], BF16, name=f"stack{h}",
                                     tag=f"stack{h}")
                    nc.vector.tensor_mul(stack, pv_ps[h][0:64, :], recip)
                    stacks[h] = stack

            # ---- proj + bias + residual ----
            out_sb = outp.tile([128, 8, C], F32, name="out_sb")
            for tci in range(8):
                pr_ps = psB.tile([128, C], F32, name="pr_ps", tag="acc")
                for h in range(H):
                    _mm(nc, pr_ps, stacks[h][:, tci * 128:(tci + 1) * 128],
                        pT[h], start=(h == 0), stop=False, r=False)
                _mm(nc, pr_ps, ones[0:1, 0:128], projb,
                    start=False, stop=True)
                nc.vector.tensor_add(out_sb[:, tci, :], pr_ps, xT_sb[:, tci, :])
                nc.sync.dma_start(
                    out=yT_d[tci * 128:(tci + 1) * 128, :], in_=out_sb[:, tci, :]
                )

    # Legalize for this walrus: at most 1 sync wait per instruction.
    import bass_rust as _bass_rust
    _bass_rust.move_matmul_waits_to_ldweights(nc.m)
    _bass_rust.generate_event_semaphores(nc)
    return nc


def host_prep(inputs):
    """Per-core input dicts (pure slicing / transpose / permutation)."""
    x = np.ascontiguousarray(np.asarray(inputs["x"], np.float32).reshape(2, C, T))
    qkv_w = np.asarray(inputs["qkv_w"], np.float32)
    proj_w = np.asarray(inputs["proj_w"], np.float32)
    norm_w = np.ascontiguousarray(np.asarray(inputs["norm_w"], np.float32))
    norm_b = np.ascontiguousarray(np.asarray(inputs["norm_b"], np.float32))
    proj_b = np.ascontiguousarray(np.asarray(inputs["proj_b"], np.float32))

    q_idx = np.concatenate([np.arange(h * 192, h * 192 + 64) for h in range(H)])
    wqT = np.ascontiguousarray(qkv_w[q_idx].T)
    wkT = np.ascontiguousarray(qkv_w[q_idx + 64].T)
    wvT = np.ascontiguousarray(qkv_w[q_idx + 128].T)
    pT = np.ascontiguousarray(proj_w.T.reshape(4, 64, C))

    sel = np.zeros((128, 16), np.float32)
    sel[np.arange(128), np.arange(128) // 8] = 1.0 / 8.0
    expand = np.zeros((16, 128), np.float32)
    expand[np.arange(128) // 8, np.arange(128)] = 1.0

    bf = __import__("ml_dtypes").bfloat16
    shared = {
        "wqT": wqT.astype(bf), "wkT": wkT.astype(bf), "wvT": wvT.astype(bf),
        "pT": pT.astype(bf),
        "normw": np.ascontiguousarray(norm_w.reshape(2, 128, 1)),
        "normb": np.ascontiguousarray(norm_b.reshape(2, 128, 1)),
        "projb": np.ascontiguousarray(proj_b.reshape(1, C)),
        "sel": sel, "expand": expand,
        "ones": np.ones((128, 128), np.float32),
        "onesb": np.ones((128, 128), np.float32).astype(
            __import__("ml_dtypes").bfloat16),
    }
    in_maps = []
    for core in range(8):
        b, i = core // 4, core % 4
        t0 = i * TS
        m = dict(shared)
        m["x"] = np.ascontiguousarray(np.roll(x[b], -t0, axis=1))
        m["xT"] = np.ascontiguousarray(x[b, :, t0:t0 + TS].T)
        in_maps.append(m)
    return in_maps


def gather(core_outs):
    y = np.empty((2, C, T), np.float32)
    for core in range(8):
        b, i = core // 4, core % 4
        y[b, :, i * TS:(i + 1) * TS] = core_outs[core].T
    return y.reshape(2, C, 16, 16, 16)


_NC = None


def _get_nc():
    global _NC
    if _NC is None:
        _NC = build_nc()
    return _NC


def run(inputs, trace=False, trace_cores=None):
    nc = _get_nc()
    in_maps = host_prep(inputs)
    res = run_bass_kernel_spmd(
        nc, in_maps, list(range(8)), trace=trace, trace_cores=trace_cores
    )
    out = gather([res.results[c]["yT"] for c in range(8)])
    return out, res


def kernel(**inputs) -> np.ndarray:
    out, _ = run(inputs)
    return out
